# revision 3
# baseline (speedup 1.0000x reference)
"""DRNN-Char (4-layer dilated QRNN + decoder) Trainium2 kernel.

Sharding: data-parallel over batch. 16 batch rows across 8 cores = 2 rows/core.
Weights replicated. Each core computes its 2 rows fully on-chip.

Layout: activations are kept feature-major [feat, time] in SBUF so that
  - gate matmuls  Y^T = W^T @ X^T  put time on the PSUM free dim
  - the fo-pool recurrence maps onto DVE tensor_tensor_scan along the free dim
  - dilated layers use strided scan APs (stride = rate), no data movement

All gate activations are computed as sigmoids on the scalar engine (a single
activation table): tanh(z) = 2*sigmoid(2z) - 1 is folded away by running the
scan in C' = (C+1)/2 space, i.e. C' = scan(sf, (1-sf)*sigmoid(2z)) with
initial carry 1/2. Sign management:
  scan' := scan(sf, (sf-1)*sigmoid(2z)) with initial -1/2  equals  -C'
  -H/2 = (scan' + 0.5) * sigmoid(o)
and the -1/2 factor is folded into the next layer's weights (W <- -2 W on the
host; likewise the decoder weight).
"""

import numpy as np
import ml_dtypes

EMB = 256
HID = 512
LAYERS = 4
VOCAB = 256
B = 16
T = 2048
NCORES = 8
BC = B // NCORES          # batch rows per core
NC512 = T // 512          # 512-wide time chunks per row
HCH = HID // 128          # hidden chunks
MCH = 3 * HCH             # m-chunks of the 3H gate output

_cache = {}


def _build():
    """Build + compile the SPMD bass program (cached across calls)."""
    if "nc" in _cache:
        return _cache["nc"]

    import concourse.bass as bass
    import concourse.mybir as mybir
    import concourse.tile as tile
    from concourse import bacc

    f32 = mybir.dt.float32
    bf16 = mybir.dt.bfloat16
    i16 = mybir.dt.int16
    SIG = mybir.ActivationFunctionType.Sigmoid
    MULT = mybir.AluOpType.mult
    ADD = mybir.AluOpType.add
    SUB = mybir.AluOpType.subtract

    nc = bacc.Bacc(
        "TRN2",
        target_bir_lowering=False,
        debug=False,
        enable_asserts=False,
        num_devices=NCORES,
    )

    # ---- DRAM parameters (per-core inputs prepared by the host) ----
    xw_d = nc.dram_tensor("xw", [128, BC * 128], i16, kind="ExternalInput").ap()
    embt_d = nc.dram_tensor("embt", [2, 128, VOCAB], f32, kind="ExternalInput").ap()
    w0_d = nc.dram_tensor("w0", [2, 128, 3 * HID], bf16, kind="ExternalInput").ap()
    w_d = [w0_d] + [
        nc.dram_tensor(f"w{i}", [4, 128, 3 * HID], bf16, kind="ExternalInput").ap()
        for i in range(1, LAYERS)
    ]
    wd_d = nc.dram_tensor("wd", [4, 128, VOCAB], bf16, kind="ExternalInput").ap()
    bias_d = nc.dram_tensor("bias", [LAYERS, 128, MCH], f32, kind="ExternalInput").ap()
    decb_d = nc.dram_tensor("decb", [128, VOCAB], f32, kind="ExternalInput").ap()
    out_d = nc.dram_tensor("out", [BC, T, VOCAB], f32, kind="ExternalOutput").ap()

    with tile.TileContext(nc) as tc:
        with (
            tc.tile_pool(name="consts", bufs=1) as consts,
            tc.tile_pool(name="acts", bufs=1) as acts,
            tc.tile_pool(name="stage", bufs=2) as stage,
            tc.tile_pool(name="x0p", bufs=2) as x0p,
            tc.tile_pool(name="outs", bufs=4) as outs,
            tc.tile_pool(name="psum", bufs=2, space="PSUM") as psum,
        ):
            # ---- resident tiles ----
            xw = consts.tile([128, BC * 128], i16, tag="xw", name="xw")
            embt = consts.tile([128, 2, VOCAB], f32, tag="embt", name="embt")
            w_sb = [consts.tile([128, 2, 3 * HID], bf16, tag="w0", name="w0")] + [
                consts.tile([128, 4, 3 * HID], bf16, tag=f"w{i}", name=f"w{i}")
                for i in range(1, LAYERS)
            ]
            wd = consts.tile([128, 4, VOCAB], bf16, tag="wd", name="wd")
            bias = consts.tile([128, LAYERS, MCH], f32, tag="bias", name="bias")
            decb = consts.tile([128, VOCAB], f32, tag="decb", name="decb")

            # ping-pong activation buffers, [128, kchunk, T] bf16, per row
            xbuf = [acts.tile([128, 4, T], bf16, tag=f"x{r}", name=f"x{r}") for r in range(BC)]
            hbuf = [acts.tile([128, 4, T], bf16, tag=f"h{r}", name=f"h{r}") for r in range(BC)]

            # ---- input DMA (ordered roughly by first use) ----
            nc.sync.dma_start(xw[:], xw_d[:])
            for e in range(2):
                nc.sync.dma_start(embt[:, e, :], embt_d[e])
            for k in range(2):
                nc.sync.dma_start(w_sb[0][:, k, :], w0_d[k])
            for li in range(LAYERS):
                nc.sync.dma_start(bias[:, li, :], bias_d[li])
            for i in range(1, LAYERS):
                for k in range(4):
                    nc.sync.dma_start(w_sb[i][:, k, :], w_d[i][k])
            for k in range(4):
                nc.sync.dma_start(wd[:, k, :], wd_d[k])
            nc.sync.dma_start(decb[:], decb_d[:])

            # ---- embedding gather: X0^T[e*128+p, t] = embT[e*128+p, x[t]] ----
            for r in range(BC):
                for e in range(2):
                    for half in range(2):
                        x0t = x0p.tile([128, 1024], f32, tag="x0", name="x0t")
                        nc.gpsimd.ap_gather(
                            x0t[:],
                            embt[:, e, :],
                            xw[:, r * 128 + half * 64 : r * 128 + (half + 1) * 64],
                            channels=128,
                            num_elems=VOCAB,
                            d=1,
                            num_idxs=1024,
                        )
                        nc.vector.tensor_copy(
                            xbuf[r][:, e, half * 1024 : (half + 1) * 1024], x0t[:]
                        )

            # ---- QRNN layers ----
            for li in range(LAYERS):
                rate = 2 ** li
                kch = 2 if li == 0 else 4
                Wt = w_sb[li]
                for r in range(BC):
                    xin, hout = xbuf[r], hbuf[r]
                    for h in range(HCH):
                        gt = {}
                        for gi, gname in enumerate(("z", "f", "o")):
                            m = gi * HCH + h
                            ps = psum.tile([128, T], f32, tag="ps", name="ps")
                            for k in range(kch):
                                for n in range(NC512):
                                    nc.tensor.matmul(
                                        ps[:, n * 512 : (n + 1) * 512],
                                        lhsT=Wt[:, k, m * 128 : (m + 1) * 128],
                                        rhs=xin[:, k, n * 512 : (n + 1) * 512],
                                        start=(k == 0),
                                        stop=(k == kch - 1),
                                    )
                            g = stage.tile([128, T], f32, tag=gname, name=gname)
                            nc.scalar.activation(
                                g[:],
                                ps[:],
                                SIG,
                                bias=bias[:, li, m : m + 1],
                                scale=2.0 if gi == 0 else 1.0,
                            )
                            gt[gname] = g
                        # z <- (f - 1) * z   (= -(1-sf)*sigmoid(2z), in place)
                        nc.vector.scalar_tensor_tensor(
                            gt["z"][:], gt["f"][:], 1.0, gt["z"][:], SUB, MULT
                        )
                        # scan' = scan(f, z') along time, stride = rate, init -1/2
                        cc = stage.tile([128, T], f32, tag="cc", name="cc")
                        for j in range(rate):
                            sl = slice(j, T, rate)
                            nc.vector.tensor_tensor_scan(
                                cc[:, sl],
                                gt["f"][:, sl],
                                gt["z"][:, sl],
                                initial=-0.5,
                                op0=MULT,
                                op1=ADD,
                            )
                        # hout_h = (scan' + 0.5) * so   (= -H/2, bf16)
                        nc.vector.scalar_tensor_tensor(
                            hout[:, h, :], cc[:], 0.5, gt["o"][:], ADD, MULT
                        )
                    xbuf[r], hbuf[r] = hbuf[r], xbuf[r]

            # ---- decoder: out[t, v] = H^T[:,t] . (-2 decW)[:, v] + decb ----
            for r in range(BC):
                xin = xbuf[r]
                for mt in range(T // 128):
                    ps = psum.tile([128, T], f32, tag="ps", name="ps")
                    for k in range(4):
                        nc.tensor.matmul(
                            ps[:, 0:VOCAB],
                            lhsT=xin[:, k, mt * 128 : (mt + 1) * 128],
                            rhs=wd[:, k, :],
                            start=(k == 0),
                            stop=(k == 3),
                        )
                    ot = outs.tile([128, VOCAB], f32, tag="ot", name="ot")
                    nc.vector.tensor_tensor(ot[:], ps[:, 0:VOCAB], decb[:], op=ADD)
                    nc.sync.dma_start(out_d[r, mt * 128 : (mt + 1) * 128, :], ot[:])

    nc.compile()
    _cache["nc"] = nc
    return nc


def _prep_inputs(inputs):
    """Host-side sharding + layout/dtype prep. Returns in_maps for 8 cores."""
    bf = ml_dtypes.bfloat16
    x = np.asarray(inputs["x"]).astype(np.int64)
    emb = np.asarray(inputs["emb"], dtype=np.float32)
    Ws = [np.asarray(inputs[f"W{i}"], dtype=np.float32) for i in range(LAYERS)]
    bs = [np.asarray(inputs[f"b{i}"], dtype=np.float32) for i in range(LAYERS)]
    decW = np.asarray(inputs["decW"], dtype=np.float32)
    decb = np.asarray(inputs["decb"], dtype=np.float32)

    embt = np.ascontiguousarray(emb.T).reshape(2, 128, VOCAB)
    w0 = Ws[0].reshape(2, 128, 3 * HID).astype(bf)
    wscaled = [(-2.0 * Ws[i]).reshape(4, 128, 3 * HID).astype(bf) for i in range(1, LAYERS)]
    wd = (-2.0 * decW).reshape(4, 128, VOCAB).astype(bf)

    bias = np.zeros((LAYERS, 128, MCH), np.float32)
    for li in range(LAYERS):
        bm = bs[li].reshape(MCH, 128).T  # [128, m]
        bias[li] = bm
        bias[li, :, :HCH] *= 2.0  # z gates: sigmoid(2z + 2b)

    decbb = np.ascontiguousarray(np.broadcast_to(decb, (128, VOCAB))).astype(np.float32)

    in_maps = []
    for c in range(NCORES):
        xw = np.zeros((128, BC * 128), np.int16)
        for r in range(BC):
            xr = x[BC * c + r].reshape(128, 16)  # [s, p%16]
            xw[:, r * 128 : (r + 1) * 128] = np.tile(xr.T, (8, 1))
        in_maps.append(
            {
                "xw": xw,
                "embt": embt,
                "w0": w0,
                "w1": wscaled[0],
                "w2": wscaled[1],
                "w3": wscaled[2],
                "wd": wd,
                "bias": bias,
                "decb": decbb,
            }
        )
    return in_maps


def kernel(**inputs) -> np.ndarray:
    from concourse.bass_utils import run_bass_kernel_spmd

    nc = _build()
    in_maps = _prep_inputs(inputs)
    res = run_bass_kernel_spmd(nc, in_maps, list(range(NCORES)))
    out = np.empty((B, T, VOCAB), np.float32)
    for c in range(NCORES):
        out[BC * c : BC * (c + 1)] = res.results[c]["out"]
    return out


# revision 5
# speedup vs baseline: 1.5622x; 1.5622x over previous
"""DRNN-Char (4-layer dilated QRNN + decoder) Trainium2 kernel.

Sharding: data-parallel over batch. 16 batch rows across 8 cores = 2 rows/core.
Weights replicated. Each core computes its 2 rows fully on-chip.

Layout: activations are kept feature-major [feat, time] in SBUF so that
  - gate matmuls  Y^T = W^T @ X^T  put time on the PSUM free dim
  - the fo-pool recurrence maps onto DVE tensor_tensor_scan along the free dim
  - dilated layers use strided scan APs (stride = rate), no data movement

Embedding: the host sends a one-hot encoding of x; layer 0's weight is the
fused table W0f = embT @ W0 (computed on device at startup with 12 small
matmuls), so  Y0^T = W0f^T @ onehot  is the embedding lookup and the layer-0
matmul in one — same matmul count as a plain layer-0, no gather needed.

All gate activations are computed as sigmoids on the scalar engine (a single
activation table): tanh(z) = 2*sigmoid(2z) - 1 is folded away by running the
scan in C' = (C+1)/2 space, i.e. C' = scan(sf, (1-sf)*sigmoid(2z)) with
initial carry 1/2. Sign management:
  scan' := scan(sf, (sf-1)*sigmoid(2z)) with initial -1/2  equals  -C'
  -H/2 = (scan' + 0.5) * sigmoid(o)
and the -1/2 factor is folded into the next layer's weights (W <- -2 W on the
host; likewise the decoder weight).
"""

import numpy as np
import ml_dtypes

EMB = 256
HID = 512
LAYERS = 4
VOCAB = 256
B = 16
T = 2048
NCORES = 8
BC = B // NCORES          # batch rows per core
NC512 = T // 512          # 512-wide time chunks per row
HCH = HID // 128          # hidden chunks
MCH = 3 * HCH             # m-chunks of the 3H gate output
GPS_PROBE = False          # clone a few DVE ops onto GpSimd, timing probe

_cache = {}


def _build():
    """Build + compile the SPMD bass program (cached across calls)."""
    if "nc" in _cache:
        return _cache["nc"]

    import concourse.bass as bass
    import concourse.mybir as mybir
    import concourse.tile as tile
    from concourse import bacc

    f32 = mybir.dt.float32
    bf16 = mybir.dt.bfloat16
    SIG = mybir.ActivationFunctionType.Sigmoid
    COPY = mybir.ActivationFunctionType.Copy
    MULT = mybir.AluOpType.mult
    ADD = mybir.AluOpType.add
    SUB = mybir.AluOpType.subtract

    nc = bacc.Bacc(
        "TRN2",
        target_bir_lowering=False,
        debug=False,
        enable_asserts=False,
        num_devices=NCORES,
    )

    # ---- DRAM parameters (per-core inputs prepared by the host) ----
    oh_d = nc.dram_tensor("oh", [BC, 2, 128, T], bf16, kind="ExternalInput").ap()
    embt_d = nc.dram_tensor("embt", [2, 128, VOCAB], bf16, kind="ExternalInput").ap()
    w0_d = nc.dram_tensor("w0", [2, 128, 3 * HID], bf16, kind="ExternalInput").ap()
    w_d = [w0_d] + [
        nc.dram_tensor(f"w{i}", [4, 128, 3 * HID], bf16, kind="ExternalInput").ap()
        for i in range(1, LAYERS)
    ]
    wd_d = nc.dram_tensor("wd", [4, 128, VOCAB], bf16, kind="ExternalInput").ap()
    bias_d = nc.dram_tensor("bias", [LAYERS, 128, MCH], f32, kind="ExternalInput").ap()
    decb_d = nc.dram_tensor("decb", [1, VOCAB], bf16, kind="ExternalInput").ap()
    out_d = nc.dram_tensor("out", [BC, T, VOCAB], f32, kind="ExternalOutput").ap()
    dbg_d = nc.dram_tensor("dbg", [128, 64], f32, kind="ExternalOutput").ap()

    with tile.TileContext(nc) as tc:
        with (
            tc.tile_pool(name="consts", bufs=1) as consts,
            tc.tile_pool(name="acts", bufs=1) as acts,
            tc.tile_pool(name="stage", bufs=2) as stage,
            tc.tile_pool(name="outs", bufs=4) as outs,
            tc.tile_pool(name="psum", bufs=2, space="PSUM") as psum,
        ):
            # ---- resident tiles ----
            embt = consts.tile([128, 2, VOCAB], bf16, tag="embt", name="embt")
            w_sb = [consts.tile([128, 2, 3 * HID], bf16, tag="w0", name="w0")] + [
                consts.tile([128, 4, 3 * HID], bf16, tag=f"w{i}", name=f"w{i}")
                for i in range(1, LAYERS)
            ]
            w0f = consts.tile([128, 2, 3 * HID], bf16, tag="w0f", name="w0f")
            wd = consts.tile([128, 4, VOCAB], bf16, tag="wd", name="wd")
            bias = consts.tile([128, LAYERS, MCH], f32, tag="bias", name="bias")
            decb = consts.tile([1, VOCAB], bf16, tag="decb", name="decb")
            ones = consts.tile([1, 128], bf16, tag="ones", name="ones")
            dbg = consts.tile([128, 64], f32, tag="dbg", name="dbg")

            # ping-pong activation buffers, [128, kchunk, T] bf16, per row
            xbuf = [acts.tile([128, 4, T], bf16, tag=f"x{r}", name=f"x{r}") for r in range(BC)]
            hbuf = [acts.tile([128, 4, T], bf16, tag=f"h{r}", name=f"h{r}") for r in range(BC)]

            # ---- input DMA (ordered roughly by first use) ----
            for e in range(2):
                nc.sync.dma_start(embt[:, e, :], embt_d[e])
            for k in range(2):
                nc.sync.dma_start(w_sb[0][:, k, :], w0_d[k])
            for e in range(2):
                nc.sync.dma_start(xbuf[0][:, e, :], oh_d[0, e])
            for li in range(LAYERS):
                nc.sync.dma_start(bias[:, li, :], bias_d[li])
            for e in range(2):
                nc.sync.dma_start(xbuf[1][:, e, :], oh_d[1, e])
            for k in range(4):
                nc.sync.dma_start(w_sb[1][:, k, :], w_d[1][k])
            nc.sync.dma_start(decb[:], decb_d[:])
            nc.gpsimd.memset(ones[:], 1.0)
            for i in range(2, LAYERS):
                for k in range(4):
                    nc.sync.dma_start(w_sb[i][:, k, :], w_d[i][k])
            for k in range(4):
                nc.sync.dma_start(wd[:, k, :], wd_d[k])

            # ---- fused layer-0 table: w0f[v, :] = (embT.T @ W0)[v, :] ----
            for m in range(2):  # vocab chunk (psum partition)
                psf = psum.tile([128, T], f32, tag="ps", name="psf")
                for k in range(2):
                    for n in range(3):
                        nc.tensor.matmul(
                            psf[:, n * 512 : (n + 1) * 512],
                            lhsT=embt[:, k, m * 128 : (m + 1) * 128],
                            rhs=w_sb[0][:, k, n * 512 : (n + 1) * 512],
                            start=(k == 0),
                            stop=(k == 1),
                        )
                for n in range(3):
                    nc.scalar.activation(
                        w0f[:, m, n * 512 : (n + 1) * 512],
                        psf[:, n * 512 : (n + 1) * 512],
                        COPY,
                    )

            # ---- QRNN layers ----
            for li in range(LAYERS):
                rate = 2 ** li
                kch = 2 if li == 0 else 4
                Wt = w0f if li == 0 else w_sb[li]
                for r in range(BC):
                    xin, hout = xbuf[r], hbuf[r]
                    for h in range(HCH):
                        gt = {}
                        for gi, gname in enumerate(("z", "f", "o")):
                            m = gi * HCH + h
                            ps = psum.tile([128, T], f32, tag="ps", name="ps")
                            for k in range(kch):
                                for n in range(NC512):
                                    nc.tensor.matmul(
                                        ps[:, n * 512 : (n + 1) * 512],
                                        lhsT=Wt[:, k, m * 128 : (m + 1) * 128],
                                        rhs=xin[:, k, n * 512 : (n + 1) * 512],
                                        start=(k == 0),
                                        stop=(k == kch - 1),
                                    )
                            g = stage.tile([128, T], f32, tag=gname, name=gname)
                            nc.scalar.activation(
                                g[:],
                                ps[:],
                                SIG,
                                bias=bias[:, li, m : m + 1],
                                scale=2.0 if gi == 0 else 1.0,
                            )
                            gt[gname] = g
                        # z <- (f - 1) * z   (= -(1-sf)*sigmoid(2z), in place)
                        nc.vector.scalar_tensor_tensor(
                            gt["z"][:], gt["f"][:], 1.0, gt["z"][:], SUB, MULT
                        )
                        # scan' = scan(f, z') along time, stride = rate, init -1/2
                        cc = stage.tile([128, T], f32, tag="cc", name="cc")
                        for j in range(rate):
                            sl = slice(j, T, rate)
                            nc.vector.tensor_tensor_scan(
                                cc[:, sl],
                                gt["f"][:, sl],
                                gt["z"][:, sl],
                                initial=-0.5,
                                op0=MULT,
                                op1=ADD,
                            )
                        # GpSimd probe: clone this h's ops on gpsimd, timing only
                        if GPS_PROBE and li == 2 and r == 0 and h == 3:
                            cc2 = stage.tile([128, T], f32, tag="cc2", name="cc2")
                            # contiguous full-row scan
                            nc.gpsimd.tensor_tensor_scan(
                                cc2[:], gt["f"][:], gt["z"][:],
                                initial=-0.5, op0=MULT, op1=ADD,
                            )
                            nc.vector.tensor_copy(dbg[:, 0:32], cc2[:, 0:32])
                            # strided scans like this layer's
                            for j in range(rate):
                                sl2 = slice(j, T, rate)
                                nc.gpsimd.tensor_tensor_scan(
                                    cc2[:, sl2], gt["f"][:, sl2], gt["z"][:, sl2],
                                    initial=-0.5, op0=MULT, op1=ADD,
                                )
                            nc.vector.tensor_copy(dbg[:, 32:64], cc2[:, 0:32])
                            nc.sync.dma_start(dbg_d[:], dbg[:])
                        # hout_h = (scan' + 0.5) * so   (= -H/2, bf16)
                        nc.vector.scalar_tensor_tensor(
                            hout[:, h, :], cc[:], 0.5, gt["o"][:], ADD, MULT
                        )
                    xbuf[r], hbuf[r] = hbuf[r], xbuf[r]

            # ---- decoder: out[t, v] = H^T[:,t] . (-2 decW)[:, v] + decb ----
            for r in range(BC):
                xin = xbuf[r]
                for mt in range(T // 128):
                    ps = psum.tile([128, T], f32, tag="ps", name="ps")
                    for k in range(4):
                        nc.tensor.matmul(
                            ps[:, 0:VOCAB],
                            lhsT=xin[:, k, mt * 128 : (mt + 1) * 128],
                            rhs=wd[:, k, :],
                            start=(k == 0),
                            stop=False,
                        )
                    nc.tensor.matmul(
                        ps[:, 0:VOCAB],
                        lhsT=ones[:],
                        rhs=decb[:],
                        start=False,
                        stop=True,
                    )
                    ot = outs.tile([128, VOCAB], f32, tag="ot", name="ot")
                    nc.scalar.activation(ot[:], ps[:, 0:VOCAB], COPY)
                    nc.sync.dma_start(out_d[r, mt * 128 : (mt + 1) * 128, :], ot[:])

    nc.compile()
    _cache["nc"] = nc
    return nc


def _prep_inputs(inputs):
    """Host-side sharding + layout/dtype prep. Returns in_maps for 8 cores."""
    bf = ml_dtypes.bfloat16
    x = np.asarray(inputs["x"]).astype(np.int64)
    emb = np.asarray(inputs["emb"], dtype=np.float32)
    Ws = [np.asarray(inputs[f"W{i}"], dtype=np.float32) for i in range(LAYERS)]
    bs = [np.asarray(inputs[f"b{i}"], dtype=np.float32) for i in range(LAYERS)]
    decW = np.asarray(inputs["decW"], dtype=np.float32)
    decb = np.asarray(inputs["decb"], dtype=np.float32)

    embt = np.ascontiguousarray(emb.T).reshape(2, 128, VOCAB).astype(bf)
    w0 = Ws[0].reshape(2, 128, 3 * HID).astype(bf)
    wscaled = [(-2.0 * Ws[i]).reshape(4, 128, 3 * HID).astype(bf) for i in range(1, LAYERS)]
    wd = (-2.0 * decW).reshape(4, 128, VOCAB).astype(bf)

    bias = np.zeros((LAYERS, 128, MCH), np.float32)
    for li in range(LAYERS):
        bm = bs[li].reshape(MCH, 128).T  # [128, m]
        bias[li] = bm
        bias[li, :, :HCH] *= 2.0  # z gates: sigmoid(2z + 2b)

    decbb = decb.reshape(1, VOCAB).astype(bf)

    in_maps = []
    for c in range(NCORES):
        oh = np.zeros((BC, VOCAB, T), bf)
        for r in range(BC):
            oh[r, x[BC * c + r], np.arange(T)] = 1.0
        in_maps.append(
            {
                "oh": oh.reshape(BC, 2, 128, T),
                "embt": embt,
                "w0": w0,
                "w1": wscaled[0],
                "w2": wscaled[1],
                "w3": wscaled[2],
                "wd": wd,
                "bias": bias,
                "decb": decbb,
            }
        )
    return in_maps


def kernel(**inputs) -> np.ndarray:
    from concourse.bass_utils import run_bass_kernel_spmd

    nc = _build()
    in_maps = _prep_inputs(inputs)
    res = run_bass_kernel_spmd(nc, in_maps, list(range(NCORES)))
    out = np.empty((B, T, VOCAB), np.float32)
    for c in range(NCORES):
        out[BC * c : BC * (c + 1)] = res.results[c]["out"]
    return out
